# revision 1
# baseline (speedup 1.0000x reference)
"""DKVMN forward kernel for Trainium2, 8-core batch-parallel, scan-based.

Model (per sample): T=200 sequential DKVMN memory steps over state
Mv [M=64, D=64], with read-before-update, plus embedding gathers and
small projections around the recurrence.

Sharding: data-parallel over batch. 64 samples -> 8 cores x 8 samples.
Parameters replicated per core.

Row space is TIME-MAJOR in 128-row blocks: block c covers timesteps
t in [c*16, (c+1)*16) for all 8 local samples:
  row = c*128 + b_loc*16 + tt,  b_loc = g*2 + s in [0,8), tt in [0,16).
13 blocks = 208 padded timesteps (t >= 200 is pad, computed, unread).
Stage-A block c feeds stage-B chunk c directly -> full pipeline overlap.

Recurrence (per core, partitions (s,d) = 128, free (g,m)=256 per step):
  S_t = S_{t-1} * (1 - w_t (x) e_t) + w_t (x) a_t ;  read_t = sum_m w*S_{t-1}
run as a *batched segmented scan*: per chunk of C=16 steps,
  GT[(s,d),(g,m,t)] = 1 - wbc*e   (w broadcast via PE matmul, fp16)
  UT              = wbc*a
  GT[...,0]=0, UT[...,0]=GT0*S_prev+UT0   (segment reset carries state)
  SALL = tensor_tensor_scan(GT, UT)       (state fp32 internal)
  P1 = SALL(shift 1) * wbc ; reads = reduce_m(P1)
"""

import numpy as np

import concourse.bass as bass
import concourse.bacc as bacc
import concourse.tile as tile
from concourse import mybir
from concourse.bass_utils import run_bass_kernel_spmd

F32 = mybir.dt.float32
F16 = mybir.dt.float16
BF16 = mybir.dt.bfloat16
I32 = mybir.dt.int32
AL = mybir.AluOpType
AF = mybir.ActivationFunctionType
AX = mybir.AxisListType

NUM_SKILLS = 1000
D = 64          # dim_s
M = 64          # size_m
B = 64          # global batch
T = 200
NCORES = 8
BL = B // NCORES          # samples per core = 8
C = 16                    # timesteps per chunk / block
NBLK = 13                 # 13 blocks of 128 rows = 208 padded steps
TP = NBLK * C             # 208
RPAD = NBLK * 128         # 1664


def _build():
    import os
    stages = os.environ.get("KSTAGES", "ABC")
    nc = bacc.Bacc(None, target_bir_lowering=False, debug=False)

    # ---- external I/O ----
    d_idxk = nc.dram_tensor("idxk", [RPAD], I32, kind="ExternalInput")
    d_resp = nc.dram_tensor("resp", [RPAD], I32, kind="ExternalInput")
    d_kemb = nc.dram_tensor("kemb", [NUM_SKILLS, D], F32, kind="ExternalInput")
    d_vemb = nc.dram_tensor("vemb", [2 * NUM_SKILLS, D], F32, kind="ExternalInput")
    d_mkt = nc.dram_tensor("mkt", [D, M], BF16, kind="ExternalInput")       # Mk^T
    d_eawt = nc.dram_tensor("eawt", [D, 2 * D], BF16, kind="ExternalInput")  # [eW^T|aW^T]
    d_fwt1 = nc.dram_tensor("fwt1", [128, D], BF16, kind="ExternalInput")   # fW^T rows 0:64, x2
    d_fwt2 = nc.dram_tensor("fwt2", [128, D], BF16, kind="ExternalInput")   # fW^T rows 64:128, x2
    d_pwc = nc.dram_tensor("pwc", [128, 1], BF16, kind="ExternalInput")     # pW col x2
    d_ebc = nc.dram_tensor("ebc", [D, 1], F32, kind="ExternalInput")        # eb col
    d_abc = nc.dram_tensor("abc", [D, 1], F32, kind="ExternalInput")        # ab col
    d_fbc = nc.dram_tensor("fbc", [D, 1], F32, kind="ExternalInput")        # fb col
    d_ind8 = nc.dram_tensor("ind8", [8, 4 * 128], F16, kind="ExternalInput")
    d_ident = nc.dram_tensor("ident", [128, 128], F32, kind="ExternalInput")
    d_s0 = nc.dram_tensor("s0", [128, 4 * M], F16, kind="ExternalInput")    # Mv0 prelaid
    d_out = nc.dram_tensor("out", [BL, T - 1], F32, kind="ExternalOutput")

    pb_host = _PB[0]

    with tile.TileContext(nc) as tc:
        import contextlib
        with contextlib.ExitStack() as ctx:
            singles = ctx.enter_context(tc.tile_pool(name="singles", bufs=1))

            t_idxk = singles.tile([128, NBLK], I32)
            t_idxv = singles.tile([128, NBLK], I32)
            t_resp = singles.tile([128, NBLK], I32)
            t_mkt = singles.tile([D, M], BF16)
            t_eawt = singles.tile([D, 2 * D], BF16)
            t_fwt1 = singles.tile([128, D], BF16)
            t_fwt2 = singles.tile([128, D], BF16)
            t_pwc = singles.tile([128, 1], BF16)
            t_ebc = singles.tile([D, 1], F32)
            t_abc = singles.tile([D, 1], F32)
            t_fbc = singles.tile([D, 1], F32)
            t_ind8 = singles.tile([8, 4 * 128], F16)
            t_ident = singles.tile([128, 128], F32)
            t_s0 = singles.tile([128, 4 * M], F16)
            t_kT = singles.tile([D, RPAD], BF16)       # k^T, block-row cols
            t_eaT = singles.tile([D, 2, RPAD], F16)    # sigmoid(e)/tanh(a) pre-shuffle
            t_EA4 = singles.tile([128, NBLK, 2, 4, C], F16)  # [(s,d), c, e|a, g, tt]
            t_WS = singles.tile([BL, M * TP], F16)     # w, [b_loc, (t,m)]
            t_reads = singles.tile([128, 4, TP], F32)
            t_rb0 = singles.tile([64, 4, TP], BF16)
            t_rb1 = singles.tile([64, 4, TP], BF16)
            t_psig = singles.tile([BL, TP], F32)
            # chunk-carried scan state (alternate buffers)
            t_sall0 = singles.tile([128, 4, M, C + 1], F16)
            t_sall1 = singles.tile([128, 4, M, C + 1], F16)
            t_sall = [t_sall0, t_sall1]

            nc.sync.dma_start(out=t_idxk[:], in_=d_idxk[:].rearrange("(c p) -> p c", p=128))
            nc.sync.dma_start(out=t_resp[:], in_=d_resp[:].rearrange("(c p) -> p c", p=128))
            nc.sync.dma_start(out=t_mkt[:], in_=d_mkt[:])
            nc.sync.dma_start(out=t_eawt[:], in_=d_eawt[:])
            nc.sync.dma_start(out=t_fwt1[:], in_=d_fwt1[:])
            nc.sync.dma_start(out=t_fwt2[:], in_=d_fwt2[:])
            nc.sync.dma_start(out=t_pwc[:], in_=d_pwc[:])
            nc.sync.dma_start(out=t_ebc[:], in_=d_ebc[:])
            nc.sync.dma_start(out=t_abc[:], in_=d_abc[:])
            nc.sync.dma_start(out=t_fbc[:], in_=d_fbc[:])
            nc.sync.dma_start(out=t_ind8[:], in_=d_ind8[:])
            nc.sync.dma_start(out=t_ident[:], in_=d_ident[:])
            nc.sync.dma_start(out=t_s0[:], in_=d_s0[:])

            # v-table index: x = skills + NUM_SKILLS * responses
            nc.vector.tensor_scalar(out=t_idxv[:], in0=t_resp[:], scalar1=NUM_SKILLS,
                                    scalar2=None, op0=AL.mult)
            nc.vector.tensor_tensor(out=t_idxv[:], in0=t_idxv[:], in1=t_idxk[:], op=AL.add)

            with tc.tile_pool(name="sa_sb", bufs=3) as sa, \
                 tc.tile_pool(name="sa_ps", bufs=1, space="PSUM") as sap, \
                 tc.tile_pool(name="sb_sb", bufs=2) as sb, \
                 tc.tile_pool(name="sb_ps", bufs=4, space="PSUM") as sbp:
                for c in range(NBLK):
                    # ============ stage A: gathers, w / e / a for block c ============
                    kg = sa.tile([128, D], F32, tag="kg")
                    vg = sa.tile([128, D], F32, tag="vg")
                    nc.gpsimd.indirect_dma_start(
                        out=kg[:], out_offset=None, in_=d_kemb[:],
                        in_offset=bass.IndirectOffsetOnAxis(ap=t_idxk[:, c:c + 1], axis=0))
                    nc.gpsimd.indirect_dma_start(
                        out=vg[:], out_offset=None, in_=d_vemb[:],
                        in_offset=bass.IndirectOffsetOnAxis(ap=t_idxv[:, c:c + 1], axis=0))
                    kTp = sap.tile([D, 128], F32, tag="ktp", space="PSUM")
                    vTp = sap.tile([D, 128], F32, tag="vtp", space="PSUM")
                    nc.tensor.transpose(out=kTp[:], in_=kg[:], identity=t_ident[:])
                    nc.tensor.transpose(out=vTp[:], in_=vg[:], identity=t_ident[:])
                    nc.scalar.copy(out=t_kT[:, c * 128:(c + 1) * 128], in_=kTp[:])
                    vT = sa.tile([D, 128], BF16, tag="vt")
                    nc.scalar.copy(out=vT[:], in_=vTp[:])

                    # logits = k @ Mk^T -> [128 rows, M]
                    lg = sap.tile([128, M], F32, tag="lg", space="PSUM")
                    nc.tensor.matmul(lg[:], lhsT=t_kT[:, c * 128:(c + 1) * 128],
                                     rhs=t_mkt[:], start=True, stop=True)
                    mx = sa.tile([128, 1], F32, tag="mx")
                    nc.vector.tensor_reduce(out=mx[:], in_=lg[:], axis=AX.X,
                                            op=AL.max, negate=True)
                    wexp = sa.tile([128, M], F32, tag="wexp")
                    sme = sa.tile([128, 1], F32, tag="sme")
                    nc.scalar.activation(out=wexp[:], in_=lg[:], func=AF.Exp,
                                         bias=mx[:], scale=1.0, accum_out=sme[:])
                    rin = sa.tile([128, 1], F32, tag="rin")
                    nc.vector.reciprocal(out=rin[:], in_=sme[:])
                    wb = sa.tile([128, M], F16, tag="wb")
                    nc.vector.tensor_scalar(out=wb[:], in0=wexp[:], scalar1=rin[:],
                                            scalar2=None, op0=AL.mult)
                    # w rows -> s-major scan layout WS[b_loc, (t, m)]
                    nc.sync.dma_start(
                        out=t_WS[:, (c * C) * M:(c * C + C) * M]
                            .rearrange("b (tt m) -> b tt m", tt=C),
                        in_=wb[:].rearrange("p m -> p m"))

                    # e/a: eaT = [eW^T|aW^T]^T @ v^T -> [(e|a)d, rows]
                    eaT = sap.tile([2 * D, 128], F32, tag="eat", space="PSUM")
                    nc.tensor.matmul(eaT[:], lhsT=t_eawt[:], rhs=vT[:],
                                     start=True, stop=True)
                    nc.scalar.activation(out=t_eaT[:, 0, c * 128:(c + 1) * 128],
                                         in_=eaT[0:D, :], func=AF.Sigmoid,
                                         bias=t_ebc[:], scale=1.0)
                    nc.scalar.activation(out=t_eaT[:, 1, c * 128:(c + 1) * 128],
                                         in_=eaT[D:2 * D, :], func=AF.Tanh,
                                         bias=t_abc[:], scale=1.0)
                    # shuffle: b_loc = s*4+g so each s-half is 64 contiguous cols
                    for s in range(2):
                        nc.sync.dma_start(
                            out=t_EA4[s * D:(s + 1) * D, c, :, :, :],
                            in_=t_eaT[:, :, c * 128 + s * 64:c * 128 + (s + 1) * 64]
                                .rearrange("p ea (g tt) -> p ea g tt", g=4))

                    # ============ stage B: chunk c (timesteps c*16 .. +16) ============
                    if "B" not in stages:
                        continue
                    wbc = sb.tile([128, 4, M, C], F16, tag="wbc")
                    for h in range(2):
                        for g in range(4):
                            wps = sbp.tile([128, 8 * M], F32, tag="wps", space="PSUM")
                            nc.tensor.matmul(
                                wps[:],
                                lhsT=t_ind8[:, g * 128:(g + 1) * 128],
                                rhs=t_WS[:, (c * C + h * 8) * M:(c * C + h * 8 + 8) * M],
                                start=True, stop=True)
                            # PSUM (t,m) -> SBUF fp16 (m,t)
                            nc.scalar.copy(
                                out=wbc[:, g, :, h * 8:(h + 1) * 8],
                                in_=wps[:].rearrange("p (t m) -> p m t", t=8))

                    ebc4 = t_EA4[:, c, 0, :, :].unsqueeze(2).broadcast_to([128, 4, M, C])
                    abc4 = t_EA4[:, c, 1, :, :].unsqueeze(2).broadcast_to([128, 4, M, C])

                    prod = sb.tile([128, 4, M, C], F16, tag="prod")
                    nc.vector.tensor_tensor(out=prod[:], in0=wbc[:], in1=ebc4, op=AL.mult)
                    gt = sb.tile([128, 4, M, C + 1], F16, tag="gt")
                    nc.scalar.activation(
                        out=gt[:, :, :, 1:C + 1].rearrange("p g m t -> p (g m) t"),
                        in_=prod[:].rearrange("p g m t -> p (g m) t"),
                        func=AF.Copy, bias=1.0, scale=-1.0)
                    nc.vector.memset(gt[:, :, :, 0], 0.0)
                    ut = sb.tile([128, 4, M, C + 1], F16, tag="ut")
                    nc.vector.tensor_tensor(out=ut[:, :, :, 1:C + 1], in0=wbc[:],
                                            in1=abc4, op=AL.mult)
                    # carry state into slot 0 (segment reset)
                    sprev = t_s0[:].rearrange("p (g m) -> p g m", g=4) if c == 0 \
                        else t_sall[(c - 1) % 2][:, :, :, C]
                    nc.scalar.copy(out=ut[:, :, :, 0], in_=sprev)

                    # the scan: S_t = GT_t * S_{t-1} + UT_t along flat (g,m,t)
                    sall = t_sall[c % 2]
                    nc.vector.tensor_tensor_scan(
                        out=sall[:].rearrange("p g m t -> p (g m t)"),
                        data0=gt[:].rearrange("p g m t -> p (g m t)"),
                        data1=ut[:].rearrange("p g m t -> p (g m t)"),
                        initial=0.0, op0=AL.mult, op1=AL.add)

                    # reads: P1_t = S_{t-1} * wbc_t ; tree-sum over m
                    p1 = sb.tile([128, 4, M, C], F16, tag="p1")
                    nc.vector.tensor_tensor(out=p1[:], in0=sall[:, :, :, 0:C],
                                            in1=wbc[:], op=AL.mult)
                    r32 = sb.tile([128, 4, 32, C], F16, tag="r32")
                    nc.vector.tensor_tensor(out=r32[:], in0=p1[:, :, 0:32, :],
                                            in1=p1[:, :, 32:64, :], op=AL.add)
                    r16 = sb.tile([128, 4, 16, C], F16, tag="r16")
                    nc.vector.tensor_tensor(out=r16[:], in0=r32[:, :, 0:16, :],
                                            in1=r32[:, :, 16:32, :], op=AL.add)
                    r8 = sb.tile([128, 4, 8, C], F16, tag="r8")
                    nc.vector.tensor_tensor(out=r8[:], in0=r16[:, :, 0:8, :],
                                            in1=r16[:, :, 8:16, :], op=AL.add)
                    r4 = sb.tile([128, 4, 4, C], F16, tag="r4")
                    nc.vector.tensor_tensor(out=r4[:], in0=r8[:, :, 0:4, :],
                                            in1=r8[:, :, 4:8, :], op=AL.add)
                    r2 = sb.tile([128, 4, 2, C], F16, tag="r2")
                    nc.vector.tensor_tensor(out=r2[:], in0=r4[:, :, 0:2, :],
                                            in1=r4[:, :, 2:4, :], op=AL.add)
                    nc.vector.tensor_tensor(
                        out=t_reads[:, :, c * C:(c + 1) * C],
                        in0=r2[:, :, 0, :], in1=r2[:, :, 1, :], op=AL.add)
                    nc.scalar.copy(out=t_rb0[:, :, c * C:(c + 1) * C],
                                   in_=t_reads[0:64, :, c * C:(c + 1) * C])
                    nc.scalar.copy(out=t_rb1[:, :, c * C:(c + 1) * C],
                                   in_=t_reads[64:128, :, c * C:(c + 1) * C])

            # ============ stage C: output head, per (s,g) ============
            with tc.tile_pool(name="sc_sb", bufs=2) as sc, \
                 tc.tile_pool(name="sc_ps", bufs=2, space="PSUM") as scp:
                if "C" not in stages:
                    nc.vector.memset(t_psig[:], 0.5)
                    if "B" not in stages:
                        nc.vector.memset(t_reads[:].rearrange("p g t -> p (g t)"), 0.0)
                for s in range(2):
                    if "C" not in stages:
                        break
                    for j in range(2):  # g-pairs {2j, 2j+1} -> b_loc {s*4+2j, +1}
                        fps = scp.tile([D, 2 * TP], F32, tag="fps", space="PSUM")
                        kslice = t_kT[:].rearrange(
                            "p (c gg w) -> p gg c w", c=NBLK, gg=8)[
                            :, s * 4 + 2 * j:s * 4 + 2 * j + 2, :, :]
                        t_rb = t_rb0 if s == 0 else t_rb1
                        nc.tensor.matmul(fps[:], lhsT=t_fwt1[0:D, :],
                                         rhs=t_rb[:, 2 * j:2 * j + 2, :],
                                         start=True, stop=False)
                        nc.tensor.matmul(fps[:], lhsT=t_fwt2[0:D, :],
                                         rhs=kslice,
                                         start=False, stop=True)
                        ft = sc.tile([D, 2 * TP], BF16, tag="ft")
                        nc.scalar.activation(out=ft[:], in_=fps[:], func=AF.Tanh,
                                             bias=t_fbc[:], scale=1.0)
                        pps = scp.tile([1, 2 * TP], F32, tag="pps", space="PSUM")
                        nc.tensor.matmul(pps[:], lhsT=t_pwc[0:D, :], rhs=ft[:],
                                         start=True, stop=True)
                        prow = sc.tile([1, 2 * TP], F32, tag="prow")
                        nc.scalar.activation(out=prow[:],
                                             in_=pps[:], func=AF.Sigmoid,
                                             bias=pb_host, scale=1.0)
                        nc.sync.dma_start(
                            out=t_psig[s * 4 + 2 * j:s * 4 + 2 * j + 2, :],
                            in_=prow[:].rearrange("p (b t) -> p b t", b=2))
                nc.sync.dma_start(out=d_out[:], in_=t_psig[:, 1:T])

    nc.compile()
    return nc


_NC_CACHE = None
_PB = [0.0]


def _get_nc():
    global _NC_CACHE
    if _NC_CACHE is None:
        _NC_CACHE = _build()
    return _NC_CACHE


def kernel(skills, responses, k_emb, v_emb, Mk, Mv0, fW, fb, eW, eb, aW, ab, pW, pb):
    skills = np.asarray(skills)
    responses = np.asarray(responses)
    k_emb = np.asarray(k_emb, dtype=np.float32)
    v_emb = np.asarray(v_emb, dtype=np.float32)
    Mk = np.asarray(Mk, dtype=np.float32)
    Mv0 = np.asarray(Mv0, dtype=np.float32)
    fW = np.asarray(fW, dtype=np.float32)
    fb = np.asarray(fb, dtype=np.float32)
    eW = np.asarray(eW, dtype=np.float32)
    eb = np.asarray(eb, dtype=np.float32)
    aW = np.asarray(aW, dtype=np.float32)
    ab = np.asarray(ab, dtype=np.float32)
    pW = np.asarray(pW, dtype=np.float32)
    pb = np.asarray(pb, dtype=np.float32)

    _PB[0] = float(pb[0])

    import ml_dtypes
    bf = ml_dtypes.bfloat16
    mkt = np.ascontiguousarray(Mk.T).astype(bf)                         # [D, M]
    eawt = np.ascontiguousarray(np.concatenate([eW.T, aW.T], axis=1)).astype(bf)
    fwt = np.ascontiguousarray(fW.T)                                    # [2D, D]
    fwt1 = np.vstack([fwt[0:D, :], fwt[0:D, :]]).astype(bf)
    fwt2 = np.vstack([fwt[D:2 * D, :], fwt[D:2 * D, :]]).astype(bf)
    pwc = np.vstack([pW.reshape(D, 1), pW.reshape(D, 1)]).astype(bf)
    ebc = np.ascontiguousarray(eb.reshape(D, 1))
    abc = np.ascontiguousarray(ab.reshape(D, 1))
    fbc = np.ascontiguousarray(fb.reshape(D, 1))
    ind8 = np.zeros((4, 8, 4, 128), np.float16)
    for g in range(4):
        for s in range(2):
            ind8[g, s * 4 + g, g, s * 64:(s + 1) * 64] = 1.0
    # layout [8, 4*128]: t_ind8[:, g*128:(g+1)*128] = selector for group g
    ind8 = np.ascontiguousarray(ind8.sum(axis=0).reshape(8, 4 * 128))
    ident = np.eye(128, dtype=np.float32)
    # S0[(s,d),(g,m)] = Mv0[m,d]
    s0 = np.tile(Mv0.T.reshape(1, D, 1, M), (2, 1, 4, 1)).reshape(128, 4 * M)
    s0 = np.ascontiguousarray(s0).astype(np.float16)

    shared = dict(kemb=k_emb, vemb=v_emb, mkt=mkt, eawt=eawt, fwt1=fwt1,
                  fwt2=fwt2, pwc=pwc, ebc=ebc, abc=abc, fbc=fbc, ind8=ind8,
                  ident=ident, s0=s0)

    in_maps = []
    for core in range(NCORES):
        sk = skills[core * BL:(core + 1) * BL].astype(np.int32)
        rs = responses[core * BL:(core + 1) * BL].astype(np.int32)
        # time-major padded layout: row = c*128 + b_loc*16 + tt, t = c*16+tt
        idxk = np.zeros((BL, TP), np.int32)
        resp = np.zeros((BL, TP), np.int32)
        idxk[:, :T] = sk
        resp[:, :T] = rs
        # [b, (c, tt)] -> [(c, b, tt)]
        idxk = idxk.reshape(BL, NBLK, C).transpose(1, 0, 2).reshape(-1)
        resp = resp.reshape(BL, NBLK, C).transpose(1, 0, 2).reshape(-1)
        m = dict(shared)
        m["idxk"] = np.ascontiguousarray(idxk)
        m["resp"] = np.ascontiguousarray(resp)
        in_maps.append(m)

    nc = _get_nc()
    res = run_bass_kernel_spmd(nc, in_maps, core_ids=list(range(NCORES)),
                               **_RUN_KWARGS)
    out = np.concatenate([res.results[i]["out"] for i in range(NCORES)], axis=0)
    global _LAST_RESULT
    _LAST_RESULT = res
    return out.astype(np.float32)


_RUN_KWARGS = {}
_LAST_RESULT = None


def run_traced(**inputs):
    """Run once with NTFF tracing; returns exec_time_ns (or None)."""
    global _RUN_KWARGS
    _RUN_KWARGS = {"trace": True}
    try:
        kernel(**inputs)
    finally:
        _RUN_KWARGS = {}
    return _LAST_RESULT.exec_time_ns if _LAST_RESULT is not None else None

